# revision 1
# baseline (speedup 1.0000x reference)
"""Multi-head attention (B=2, N=2048, C=1024, H=16) on 8 TRN2 NeuronCores.

Sharding: tensor-parallel over heads (2 heads/core) for qkv+attention,
then AllToAll to token-shard the output projection.

Per-core dataflow (bf16 matmuls, fp32 softmax-normalizer chain):
  x^T[b] (host-pretransposed) --matmul w/ W^T shards--> Q^T,K^T [128,N]
  V computed directly in [tok, ch] layout (bf16 full-rate at free dim 128)
  S^T[nk,nq] = K^T.T-slices @ Q^T  (d=64 contraction, 2 heads row-packed:
               bf16 pairs run concurrently in disjoint PE row groups)
  P^T = exp(0.125*S^T)  (ScalarE, scale folded; unsafe softmax - scores ~N(0,1))
  h_aug^T[65,nq] = [V|1].T @ P^T   (row 64 = softmax denominator, free)
  h^T = h_aug^T[0:64] * bcast(1/h_aug^T[64])
        (DVE reciprocal + GPSIMD partition_broadcast + DVE multiply)
  AllToAll per qb-pair (1024-token chunks) -> full-channel h^T slices
  out = h^T.T @ proj_w^T + b  (K=1 bf16 ones matmul adds bias into psum)

Both batches' qkv chains are emitted before either batch's attention so the
PE always has ready filler work and the ScalarE exp stream stays saturated.
"""

import numpy as np
import ml_dtypes

import concourse.mybir as mybir
import concourse.tile as tile
from concourse import bacc
from concourse.bass_utils import run_bass_kernel_spmd

F32 = mybir.dt.float32
F32R = mybir.dt.float32r
BF16 = mybir.dt.bfloat16
EXP = mybir.ActivationFunctionType.Exp

N_CORES = 8
B = 2
C = 1024
H = 16
D = 64
HPC = H // N_CORES          # heads per core
CH = HPC * D                # channels per core (128)
KT_C = C // 128             # contraction tiles (8)


def build_program(N=2048, n_cores=N_CORES, mm_dt=BF16):
    """Build the SPMD Bass program (same on every core)."""
    assert N % 512 == 0
    QB = N // 512            # 512-wide query-column blocks per batch
    NK = N // 128            # 128-row key tiles per batch
    TG = NK // 2             # key-tile pairs (exp granularity 1024)
    scale = float(D) ** -0.5
    # qb-pair groups: each becomes one AllToAll + proj chunk
    QGRP = [list(range(g, min(g + 2, QB))) for g in range(0, QB, 2)]
    if len(QGRP) > 1:
        QGRP_LAST = QGRP[:-1] + [[g] for g in QGRP[-1]]
    else:
        QGRP_LAST = QGRP
    TOKG = [512 * len(g) // n_cores for g in QGRP]   # tokens/core per group

    nc = bacc.Bacc("TRN2", target_bir_lowering=False, debug=False,
                   num_devices=n_cores)

    xT_d = nc.dram_tensor("xT", [B, C, N], mm_dt, kind="ExternalInput")
    wqT_d = nc.dram_tensor("wqT", [128, KT_C * CH], mm_dt, kind="ExternalInput")
    wkT_d = nc.dram_tensor("wkT", [128, KT_C * CH], mm_dt, kind="ExternalInput")
    wvT_d = nc.dram_tensor("wvT", [128, KT_C * CH], mm_dt, kind="ExternalInput")
    pwT_d = nc.dram_tensor("pwT", [128, KT_C * C], mm_dt, kind="ExternalInput")
    onesb_d = nc.dram_tensor("onesb", [1, 128], mm_dt, kind="ExternalInput")
    pbb_d = nc.dram_tensor("pbb", [C], mm_dt, kind="ExternalInput")
    vones_d = nc.dram_tensor("vonesc", [128, NK], mm_dt, kind="ExternalInput")
    # out[b, t0:t0+tokg, :] = this core's output tokens (flat per batch)
    TOKB = N // n_cores
    out_d = nc.dram_tensor("out", [B, TOKB, C], F32, kind="ExternalOutput")

    lp = nc.allow_low_precision("bf16 matmul pipeline")

    with tile.TileContext(nc) as tc:
        with (tc.tile_pool(name="sb", bufs=1) as sb,
              tc.tile_pool(name="ps", bufs=1, space="PSUM") as ps,
              tc.tile_pool(name="dr", bufs=1, space="DRAM") as dr,
              lp):
            # PSUM (8 banks): sst 2x2 + hav0 + hav1 + acc(qkv/v/bcp/warm) + proj

            # ---- constants (host-fed) ----
            onesb = sb.tile([1, 128], mm_dt, tag="onesb", bufs=1)
            nc.sync.dma_start(onesb[:], onesb_d.ap())
            pbb_sb = sb.tile([1, C], mm_dt, tag="pbb", bufs=1)
            nc.sync.dma_start(pbb_sb[:], pbb_d.ap().unsqueeze(0))
            vones = sb.tile([128, NK], mm_dt, tag="vones", bufs=1)
            nc.sync.dma_start(vones[:], vones_d.ap())

            wq, wk, wv, pw = [], [], [], []
            for lst, dram, nm in ((wq, wqT_d, "wq"), (wk, wkT_d, "wk"),
                                  (wv, wvT_d, "wv")):
                    wt = sb.tile([128, KT_C * CH], mm_dt, tag=nm, bufs=1,
                                 name=nm)
                    nc.sync.dma_start(wt[:], dram.ap())
                    for k in range(KT_C):
                        lst.append(wt[:, CH * k:CH * k + CH])


            state = [([], [], []) for _ in range(B)]

            NH = 2 if QB >= 2 else 1     # column halves of x^T
            HWC = 512 * QB // NH         # columns per half

            def qkv_gen(b):
                kts, qt, vau = state[b]
                # two column-halves per k-tile, emitted half-outer: the
                # first attention group's inputs land in half the DMA time
                xth = [[] for _ in range(NH)]
                for hx in range(NH):
                    for k in range(KT_C):
                        t = sb.tile([128, HWC], mm_dt, tag="xt",
                                    bufs=NH * KT_C + 1,
                                    name=f"xt{b}_{hx}_{k}")
                        nc.sync.dma_start(
                            t[:], xT_d.ap()[b, 128 * k:128 * k + 128,
                                            HWC * hx:HWC * hx + HWC])
                        xth[hx].append(t)

                def xs(k, col0, w):
                    hx = col0 // HWC
                    lo = col0 - HWC * hx
                    return xth[hx][k][:, lo:lo + w]
                for h in range(2):
                    t = sb.tile([128, 65 * NK], mm_dt, tag=f"vau{h}", bufs=2,
                                name=f"vau{b}_{h}")
                    nc.sync.dma_start(t[:, 64::65], vones_d.ap())
                    vau.append(t)

                def k_chain(qb):
                    cs = slice(512 * qb, 512 * qb + 512)
                    acc = ps.tile([128, 512], F32, tag="acc", bufs=1,
                                  name=f"kacc{b}_{qb}")
                    for k in range(KT_C):
                        nc.tensor.matmul(acc[:], wk[k], xs(k, 512 * qb, 512),
                                         start=(k == 0), stop=(k == KT_C - 1))
                        yield
                    ktile = sb.tile([128, 512], mm_dt, tag="kt", bufs=9,
                                    name=f"kt{b}_{qb}")
                    nc.vector.tensor_copy(ktile[:], acc[:])
                    kts.append(ktile)

                def q_chain(qb):
                    cs = slice(512 * qb, 512 * qb + 512)
                    acc = ps.tile([128, 512], F32, tag="acc", bufs=1,
                                  name=f"qacc{b}_{qb}")
                    for k in range(KT_C):
                        nc.tensor.matmul(acc[:], wq[k], xs(k, 512 * qb, 512),
                                         start=(k == 0), stop=(k == KT_C - 1))
                        yield
                    qtile = sb.tile([128, 512], mm_dt, tag="qt", bufs=9,
                                    name=f"qt{b}_{qb}")
                    nc.vector.tensor_copy(qtile[:], acc[:])
                    qt.append(qtile)

                for qb in range(QB):
                    yield from k_chain(qb)
                yield from q_chain(0)
                if QB > 1:
                    yield from q_chain(1)
                # V directly in [tok, ch] layout (bf16: full rate at N=128)
                for tt in range(NK):
                    ts_ = slice(128 * tt, 128 * tt + 128)
                    acc = ps.tile([128, 128], F32, tag="acc", bufs=1,
                                  name=f"vacc{b}_{tt}")
                    for k in range(KT_C):
                        nc.tensor.matmul(acc[:], xs(k, 128 * tt, 128), wv[k],
                                         start=(k == 0), stop=(k == KT_C - 1))
                        yield
                    nc.vector.tensor_copy(
                        vau[0][:, 65 * tt:65 * tt + 64], acc[:, 0:64])
                    nc.vector.tensor_copy(
                        vau[1][:, 65 * tt:65 * tt + 64], acc[:, 64:128])
                for qb in range(2, QB):
                    yield from q_chain(qb)

            def pull(g, n):
                if g is None:
                    return
                for _ in range(n):
                    if next(g, "done") == "done":
                        return

            gens = [qkv_gen(b) for b in range(B)]
            pull(gens[0], 10 ** 9)  # first batch's qkv emitted up front

            for b in range(B):
                kts, qt, vau = state[b]
                filler = gens[b + 1] if b + 1 < B else None
                # ---- attention, grouped by qb-pairs for chunked AllToAll ----
                grps = QGRP_LAST if b == B - 1 else QGRP
                a2a_outs = []
                for gi, grp in enumerate(grps):
                    tokg = 512 * len(grp) // n_cores
                    a2a_in = dr.tile([n_cores, CH, tokg], mm_dt,
                                     tag="a2ain", bufs=3, name=f"a2ain{b}_{gi}")
                    hcps = []
                    for p, qb in enumerate(grp):
                        hav = [ps.tile([65, 512], F32, tag=f"hav{h}", bufs=1,
                                       name=f"hav{b}_{qb}_{h}")
                               for h in range(2)]
                        for t_idx in range(NK):
                            # one tile per key-tile: [h0 | h1] halves, so the
                            # two S^T matmuls alternate PE row groups (paired)
                            sst = ps.tile([128, 1024], F32, tag="sst", bufs=2,
                                          name=f"sst{b}_{qb}_{t_idx}")
                            pt = sb.tile([128, 1024], mm_dt, tag="pt", bufs=8,
                                         name=f"pt{b}_{qb}_{t_idx}")
                            kb, ko = t_idx // 4, t_idx % 4
                            ks = slice(128 * ko, 128 * ko + 128)
                            for h in range(2):
                                hs = slice(64 * h, 64 * h + 64)
                                nc.tensor.matmul(
                                    sst[:, 512 * h:512 * h + 512],
                                    kts[kb][hs, ks], qt[qb][hs, :],
                                    start=True, stop=True)
                            nc.scalar.activation(pt[:], sst[:], EXP,
                                                 scale=scale)
                            for h in range(2):
                                nc.tensor.matmul(
                                    hav[h][:],
                                    vau[h][:, 65 * t_idx:65 * t_idx + 65],
                                    pt[:, 512 * h:512 * h + 512],
                                    start=(t_idx == 0),
                                    stop=(t_idx == NK - 1))
                            # interleave a few of the next batch's qkv matmuls
                            pull(filler, 5)
                        # copy h_aug out of PSUM fast so hav slots recycle
                        hcp = [sb.tile([65, 512], F32, tag=f"hcp{h}", bufs=3,
                                       name=f"hcp{b}_{qb}_{h}")
                               for h in range(2)]
                        for h in range(2):
                            nc.vector.tensor_copy(hcp[h][:], hav[h][:])
                        hcps.append(hcp)
                    for p, qb in enumerate(grp):
                        hcp = hcps[p]
                        ht = sb.tile([128, 512], mm_dt, tag="ht", bufs=2,
                                     name=f"ht{b}_{qb}")
                        for h in range(2):
                            nrr = sb.tile([1, 512], F32, tag=f"nrr{h}",
                                          bufs=2, name=f"nrr{b}_{qb}_{h}")
                            nc.vector.reciprocal(nrr[:], hcp[h][64:65, :])
                            bcs = sb.tile([64, 512], F32, tag=f"bcs{h}",
                                          bufs=2, name=f"bcs{b}_{qb}_{h}")
                            nc.gpsimd.partition_broadcast(bcs[:], nrr[:])
                            nc.vector.tensor_mul(ht[64 * h:64 * h + 64, :],
                                                 hcp[h][0:64, :], bcs[:])
                        # scatter this qb's tokens into the group A2A buffer
                        npr = 512 // tokg
                        for s in range(npr):
                            j = p * npr + s
                            nc.sync.dma_start(
                                a2a_in[j][:, 0:tokg],
                                ht[:, tokg * s:tokg * s + tokg])

                    # ---- AllToAll: head-sharded -> token-sharded ----
                    a2a_out = dr.tile([n_cores, CH, tokg], mm_dt,
                                      tag="a2aout", bufs=3,
                                      name=f"a2aout{b}_{gi}")
                    nc.gpsimd.collective_compute(
                        "AllToAll", mybir.AluOpType.bypass,
                        replica_groups=[list(range(n_cores))],
                        ins=[a2a_in.opt()], outs=[a2a_out.opt()])
                    a2a_outs.append((a2a_out, tokg))

                # ---- phase B: projections, emitted after ALL collectives so
                # collective-gated pl reads never head-of-line-block the next
                # group's scatter DMAs on the sync queue ----
                tok0 = 0
                for gi, (a2a_out, tokg) in enumerate(a2a_outs):
                    if not pw:
                        pwt = sb.tile([128, KT_C * C], mm_dt, tag="pw",
                                      bufs=1, name="pw")
                        nc.sync.dma_start(pwt[:], pwT_d.ap())
                        for k in range(KT_C):
                            pw.append(pwt[:, C * k:C * k + C])
                    pl = []
                    for k in range(KT_C):
                        t = sb.tile([128, tokg], mm_dt, tag=f"pl{k}", bufs=2,
                                    name=f"pl{b}_{gi}_{k}")
                        nc.sync.dma_start(t[:], a2a_out[k])
                        pl.append(t)
                    for oh in range(2):
                        os_ = slice(512 * oh, 512 * oh + 512)
                        acc = ps.tile([128, 512], F32, tag="proj", bufs=1,
                                      name=f"pacc{b}_{gi}_{oh}")
                        for k in range(KT_C):
                            nc.tensor.matmul(
                                acc[0:tokg, :], pl[k][:], pw[k][:, os_],
                                start=(k == 0), stop=False)
                        nc.tensor.matmul(acc[0:tokg, :], onesb[0:1, 0:tokg],
                                         pbb_sb[0:1, os_],
                                         start=False, stop=True)
                        osb = sb.tile([128, 512], F32, tag="osb", bufs=2,
                                      name=f"osb{b}_{gi}_{oh}")
                        nc.vector.tensor_copy(osb[0:tokg, :], acc[0:tokg, :])
                        nc.sync.dma_start(
                            out_d.ap()[b, tok0:tok0 + tokg, os_],
                            osb[0:tokg, :])
                    tok0 += tokg
                pull(filler, 10 ** 9)

    nc.compile()
    return nc


def shard_inputs(x, qkv_w, proj_w, proj_b, n_cores=N_CORES, mm_dt=BF16):
    """Host-side sharding: pre-transpose activations/weights, slice heads."""
    npdt = ml_dtypes.bfloat16 if mm_dt == BF16 else np.float32
    xT = np.ascontiguousarray(
        np.transpose(np.asarray(x), (0, 2, 1))).astype(npdt)
    qkv_w = np.asarray(qkv_w)

    def pack(wT):  # [C, cols] -> [128, (C//128)*cols], k-tiles side by side
        cdim, cols = wT.shape
        return np.ascontiguousarray(
            wT.reshape(cdim // 128, 128, cols).transpose(1, 0, 2)
            .reshape(128, -1)).astype(npdt)

    pwT = pack(np.asarray(proj_w).T)
    pb = np.ascontiguousarray(np.asarray(proj_b)).astype(np.float32)
    nk = x.shape[1] // 128
    vonesc = np.ones((128, nk), dtype=npdt)
    in_maps = []
    for i in range(n_cores):
        cs = slice(CH * i, CH * i + CH)
        in_maps.append({
            "xT": xT,
            "wqT": pack(qkv_w[cs, :].T),
            "wkT": pack(qkv_w[C:][cs, :].T),
            "wvT": pack(qkv_w[2 * C:][cs, :].T),
            "pwT": pwT,
            "onesb": np.ones((1, 128), dtype=npdt),
            "pbb": pb.astype(npdt),
            "vonesc": vonesc,
        })
    return in_maps


def assemble_output(res, N, n_cores=N_CORES):
    QB = N // 512
    QGRP = [list(range(g, min(g + 2, QB))) for g in range(0, QB, 2)]
    if len(QGRP) > 1:
        QGRP_LAST = QGRP[:-1] + [[g] for g in QGRP[-1]]
    else:
        QGRP_LAST = QGRP
    out = np.empty((B, N, C), dtype=np.float32)
    for i in range(n_cores):
        o = res.results[i]["out"]  # [B, TOKB, C]
        for b in range(B):
            grps = QGRP_LAST if b == B - 1 else QGRP
            tok0 = 0
            base = 0
            for grp in grps:
                tokg = 512 * len(grp) // n_cores
                lo = base + tokg * i
                out[b, lo:lo + tokg, :] = o[b, tok0:tok0 + tokg]
                tok0 += tokg
                base += 512 * len(grp)
    return out


_NC_CACHE = {}


def _get_program(N, mm_dt=BF16):
    key = (N, str(mm_dt))
    if key not in _NC_CACHE:
        _NC_CACHE[key] = build_program(N=N, mm_dt=mm_dt)
    return _NC_CACHE[key]


def kernel(x, qkv_w, proj_w, proj_b):
    x = np.asarray(x)
    Bx, N, Cx = x.shape
    assert (Bx, Cx) == (B, C), (Bx, Cx)
    nc = _get_program(N)
    in_maps = shard_inputs(x, qkv_w, proj_w, proj_b)
    res = run_bass_kernel_spmd(nc, in_maps, list(range(N_CORES)))
    return assemble_output(res, N)



# revision 12
# speedup vs baseline: 1.1130x; 1.1130x over previous
"""Multi-head attention (B=2, N=2048, C=1024, H=16) on 8 TRN2 NeuronCores.

Sharding: tensor-parallel over heads (2 heads/core) for qkv+attention,
then AllToAll to token-shard the output projection.

v2: one globally software-pipelined schedule built around saturating the
Scalar (activation) engine, which has ~142us of exp work - the hard
bottleneck of this shard layout:
  - attention iterations (sst pair -> exp -> hav pair) start as soon as
    K(0)/Q(0)/V(0..3) exist (~8us), not after the full qkv phase
  - all remaining qkv matmuls (own batch, next batch) and the output
    projections are interleaved into the iteration stream as PE filler,
    budgeted so the PE always has work (keeps its p-state ramped at
    2.4GHz) but never starves ScalarE of sst inputs
  - sst for iteration i+1 is emitted before hav for iteration i so the
    PE never waits on exp latency
  - reciprocal_approx_fast instead of reciprocal (5x) for the softmax
    denominators; normalize chain kept off the critical path
  - per-qb scatter into the AllToAll buffer and per-group gather out of
    it are single strided DMAs (transposed APs) instead of 8-16 small
    serialized DMAs
  - PE warmup matmuls at t=0 and across the final AllToAll wait keep the
    p-state up where there is no real work to overlap
"""

import numpy as np
import ml_dtypes

import concourse.mybir as mybir
import concourse.tile as tile
from concourse import bacc
from concourse.bass_utils import run_bass_kernel_spmd

F32 = mybir.dt.float32
BF16 = mybir.dt.bfloat16
EXP = mybir.ActivationFunctionType.Exp

N_CORES = 8
B = 2
C = 1024
H = 16
D = 64
HPC = H // N_CORES          # heads per core
CH = HPC * D                # channels per core (128)
KT_C = C // 128             # contraction tiles (8)

RELEASE_FIRST = 26          # iters before first collective's proj releases
RELEASE_REST = 13           # iters for subsequent collectives
FILL_PER_ITER = 2.2         # filler matmuls per attention iteration


def groups_for(b, QB):
    g = [list(range(i, min(i + 2, QB))) for i in range(0, QB, 2)]
    if b == B - 1 and len(g) > 1:
        g = g[:-1] + [[q] for q in g[-1]]
    return g


def build_program(N=2048, n_cores=N_CORES, mm_dt=BF16):
    assert N % 512 == 0
    QB = N // 512            # 512-wide query/column blocks per batch
    NK = N // 128            # 128-row key tiles per batch
    scale = float(D) ** -0.5
    TOKB = N // n_cores

    nc = bacc.Bacc("TRN2", target_bir_lowering=False, debug=False,
                   num_devices=n_cores)

    # x host-packed per 512-column block: xR[b, cb, r, k*512+c]
    xR_d = nc.dram_tensor("xR", [B, QB, 128, KT_C * 512], mm_dt,
                          kind="ExternalInput")
    wqT_d = nc.dram_tensor("wqT", [128, KT_C * CH], mm_dt, kind="ExternalInput")
    wkT_d = nc.dram_tensor("wkT", [128, KT_C * CH], mm_dt, kind="ExternalInput")
    wvT_d = nc.dram_tensor("wvT", [128, KT_C * CH], mm_dt, kind="ExternalInput")
    pwT_d = nc.dram_tensor("pwT", [128, KT_C * C], mm_dt, kind="ExternalInput")
    onesb_d = nc.dram_tensor("onesb", [1, 128], mm_dt, kind="ExternalInput")
    pbb_d = nc.dram_tensor("pbb", [C], mm_dt, kind="ExternalInput")
    vones_d = nc.dram_tensor("vonesc", [128, NK], mm_dt, kind="ExternalInput")
    out_d = nc.dram_tensor("out", [B, TOKB, C], F32, kind="ExternalOutput")

    lp = nc.allow_low_precision("bf16 matmul pipeline")

    with tile.TileContext(nc) as tc:
        with (tc.tile_pool(name="sb", bufs=1) as sb,
              tc.tile_pool(name="ps", bufs=1, space="PSUM") as ps,
              tc.tile_pool(name="dr", bufs=1, space="DRAM") as dr,
              lp):
            # PSUM budget (8 banks): sst 2x2 + hav0 + hav1 + acc 2 (shared
            # by qkv chains, projections, warmup)

            # ---- weights / constants; DMA order: wk then x(0,0) first so
            # the first K chain can start ~4us in ----
            wkt = sb.tile([128, KT_C * CH], mm_dt, tag="wk", bufs=1, name="wk")
            nc.sync.dma_start(wkt[:], wkT_d.ap())

            xt = {}

            def load_x(b, cb):
                t = sb.tile([128, KT_C * 512], mm_dt, tag="xt", bufs=2 * QB,
                            name=f"x{b}_{cb}")
                nc.sync.dma_start(t[:], xR_d.ap()[b, cb])
                xt[(b, cb)] = t

            load_x(0, 0)
            wqt = sb.tile([128, KT_C * CH], mm_dt, tag="wq", bufs=1, name="wq")
            nc.sync.dma_start(wqt[:], wqT_d.ap())
            wvt = sb.tile([128, KT_C * CH], mm_dt, tag="wv", bufs=1, name="wv")
            nc.sync.dma_start(wvt[:], wvT_d.ap())
            onesb = sb.tile([1, 128], mm_dt, tag="onesb", bufs=1)
            nc.sync.dma_start(onesb[:], onesb_d.ap())
            pbb_sb = sb.tile([1, C], mm_dt, tag="pbb", bufs=1)
            nc.sync.dma_start(pbb_sb[:], pbb_d.ap().unsqueeze(0))
            # vau ones-stripes early: the very first hav reads them
            vau = [[] for _ in range(B)]
            for vb in range(B):
                for h in range(2):
                    t = sb.tile([128, 65 * NK], mm_dt, tag=f"vau{h}", bufs=2,
                                name=f"vau{vb}_{h}")
                    nc.sync.dma_start(t[:, 64::65], vones_d.ap())
                    vau[vb].append(t)
            for cb in range(1, QB):
                load_x(0, cb)
            for cb in range(QB):
                load_x(1, cb)
            pwt = sb.tile([128, KT_C * C], mm_dt, tag="pw", bufs=1, name="pw")
            nc.sync.dma_start(pwt[:], pwT_d.ap())

            wk = [wkt[:, CH * k:CH * k + CH] for k in range(KT_C)]
            wq = [wqt[:, CH * k:CH * k + CH] for k in range(KT_C)]
            wv = [wvt[:, CH * k:CH * k + CH] for k in range(KT_C)]
            pw = [pwt[:, C * k:C * k + C] for k in range(KT_C)]

            # ---- PE warmup: ramp the p-state before real work lands ----
            def warmup(n):
                wacc = ps.tile([128, 512], F32, tag="acc", bufs=2,
                               name="warm")
                for i in range(n):
                    nc.tensor.matmul(wacc[:], wkt[:, 0:128], wkt[:, 0:512],
                                     start=(i == 0), stop=(i == n - 1))

            warmup(8)

            # ---- per-batch qkv generators with milestone tracking ----
            kts = [{} for _ in range(B)]
            qts = [{} for _ in range(B)]
            done = [set() for _ in range(B)]

            def qkv_gen(b):
                def xs(cb, k, lo, w):
                    return xt[(b, cb)][:, 512 * k + lo:512 * k + lo + w]

                def kq_chain(which, qb, wlist, store, tag):
                    acc = ps.tile([128, 512], F32, tag="acc", bufs=2,
                                  name=f"{which}acc{b}_{qb}")
                    for k in range(KT_C):
                        nc.tensor.matmul(acc[:], wlist[k], xs(qb, k, 0, 512),
                                         start=(k == 0), stop=(k == KT_C - 1))
                        yield 1
                    tgt = sb.tile([128, 512], mm_dt, tag=tag, bufs=2 * QB + 1,
                                  name=f"{tag}{b}_{qb}")
                    nc.vector.tensor_copy(tgt[:], acc[:])
                    store[qb] = tgt
                    done[b].add((which, qb))

                def v_chain(tt):
                    cb, off = tt // 4, (tt % 4) * 128
                    acc = ps.tile([128, 128], F32, tag="acc", bufs=2,
                                  name=f"vacc{b}_{tt}")
                    for k in range(KT_C):
                        nc.tensor.matmul(acc[:], xs(cb, k, off, 128), wv[k],
                                         start=(k == 0), stop=(k == KT_C - 1))
                        yield 1
                    nc.vector.tensor_copy(
                        vau[b][0][:, 65 * tt:65 * tt + 64], acc[:, 0:64])
                    nc.vector.tensor_copy(
                        vau[b][1][:, 65 * tt:65 * tt + 64], acc[:, 64:128])
                    done[b].add(('V', tt))

                yield from kq_chain('K', 0, wk, kts[b], "kt")
                yield from kq_chain('Q', 0, wq, qts[b], "qt")
                for tt in range(4):
                    yield from v_chain(tt)
                for cb in range(1, QB):
                    yield from kq_chain('K', cb, wk, kts[b], "kt")
                    for tt in range(4 * cb, 4 * cb + 4):
                        yield from v_chain(tt)
                for qb in range(1, QB):
                    yield from kq_chain('Q', qb, wq, qts[b], "qt")

            gens = [qkv_gen(b) for b in range(B)]

            units = [0]          # PE matmuls emitted this iteration

            def force(b, key):
                while key not in done[b]:
                    if next(gens[b], None) is None:
                        if key in done[b]:
                            break
                        raise RuntimeError(f"gen {b} dry before {key}")
                    units[0] += 1

            # ---- filler pool (priority order) + proj release queue ----
            fillq = [gens[1]] if B > 1 else []
            proj_ripe = []       # (release_at_iter, generator)
            it_counter = [0]

            def fill_pull():
                while fillq:
                    if next(fillq[0], None) is None:
                        fillq.pop(0)
                        continue
                    return True
                return False

            # ---- projection job per (batch, group) ----
            def proj_job(b, a2a_out, tokg, tok0, gi):
                pl = sb.tile([128, KT_C * tokg], mm_dt, tag="pl", bufs=2,
                             name=f"pl{b}_{gi}")
                nc.sync.dma_start(pl.rearrange("p (k t) -> p k t", k=KT_C),
                                  a2a_out.transpose([1, 0, 2]))
                for oh in range(2):
                    os_ = slice(512 * oh, 512 * oh + 512)
                    acc = ps.tile([128, 512], F32, tag="acc", bufs=2,
                                  name=f"pacc{b}_{gi}_{oh}")
                    for k in range(KT_C):
                        nc.tensor.matmul(
                            acc[0:tokg, :], pl[:, tokg * k:tokg * k + tokg],
                            pw[k][:, os_], start=(k == 0), stop=False)
                        yield 1
                    nc.tensor.matmul(acc[0:tokg, :], onesb[0:1, 0:tokg],
                                     pbb_sb[0:1, os_], start=False, stop=True)
                    yield 1
                    osb = sb.tile([128, 512], F32, tag="osb", bufs=2,
                                  name=f"osb{b}_{gi}_{oh}")
                    nc.vector.tensor_copy(osb[0:tokg, :], acc[0:tokg, :])
                    nc.sync.dma_start(
                        out_d.ap()[b, tok0:tok0 + tokg, os_], osb[0:tokg, :])

            # ---- attention building blocks ----
            hav = {}             # h -> current psum tile
            n_coll = [0]

            def emit_sst_exp(b, qb, t):
                force(b, ('K', t // 4))
                force(b, ('Q', qb))
                sst = ps.tile([128, 1024], F32, tag="sst", bufs=2,
                              name=f"sst{b}_{qb}_{t}")
                pt = sb.tile([128, 1024], mm_dt, tag="pt", bufs=8,
                             name=f"pt{b}_{qb}_{t}")
                kb, ko = t // 4, t % 4
                ks = slice(128 * ko, 128 * ko + 128)
                for h in range(2):
                    hs = slice(64 * h, 64 * h + 64)
                    nc.tensor.matmul(sst[:, 512 * h:512 * h + 512],
                                     kts[b][kb][hs, ks], qts[b][qb][hs, :],
                                     start=True, stop=True)
                units[0] += 1      # pair runs concurrently: one PE slot
                nc.scalar.activation(pt[:], sst[:], EXP, scale=scale)
                return pt

            grp_state = {}       # (b, gi) -> dict with a2a_in etc.

            def emit_hav(b, qb, t, pt):
                force(b, ('V', t))
                if t == 0:
                    for h in range(2):
                        hav[h] = ps.tile([65, 512], F32, tag=f"hav{h}",
                                         bufs=1, name=f"hav{b}_{qb}_{h}")
                for h in range(2):
                    nc.tensor.matmul(hav[h][:],
                                     vau[b][h][:, 65 * t:65 * t + 65],
                                     pt[:, 512 * h:512 * h + 512],
                                     start=(t == 0), stop=(t == NK - 1))
                units[0] += 2
                if t == NK - 1:
                    finish_qb(b, qb)

            def finish_qb(b, qb):
                grps = groups_for(b, QB)
                gi = next(i for i, g in enumerate(grps) if qb in g)
                grp = grps[gi]
                p = grp.index(qb)
                tokg = 512 * len(grp) // n_cores
                npr = 512 // tokg
                st = grp_state.setdefault((b, gi), {})
                if "a2a_in" not in st:
                    st["a2a_in"] = dr.tile([n_cores, CH, tokg], mm_dt,
                                           tag="a2ain", bufs=5,
                                           name=f"a2ain{b}_{gi}")
                # normalize: h/denom with approx reciprocal, then one
                # strided scatter DMA into the group's AllToAll buffer
                ht = sb.tile([128, 512], mm_dt, tag="ht", bufs=2,
                             name=f"ht{b}_{qb}")
                for h in range(2):
                    # denominator row to a partition-0 tile: the custom-DVE
                    # approx reciprocal mishandles base_partition != 0
                    dnm = sb.tile([1, 512], F32, tag=f"dnm{h}", bufs=2,
                                  name=f"dnm{b}_{qb}_{h}")
                    nc.vector.tensor_copy(dnm[:], hav[h][64:65, :])
                    hcp = sb.tile([64, 512], F32, tag=f"hcp{h}", bufs=2,
                                  name=f"hcp{b}_{qb}_{h}")
                    nc.vector.tensor_copy(hcp[:], hav[h][0:64, :])
                    nrr = sb.tile([1, 512], F32, tag=f"nrr{h}", bufs=2,
                                  name=f"nrr{b}_{qb}_{h}")
                    nc.vector.reciprocal_approx_fast(nrr[:], dnm[:])
                    bcs = sb.tile([64, 512], F32, tag=f"bcs{h}", bufs=2,
                                  name=f"bcs{b}_{qb}_{h}")
                    nc.gpsimd.partition_broadcast(bcs[:], nrr[:])
                    nc.vector.tensor_mul(ht[64 * h:64 * h + 64, :],
                                         hcp[:], bcs[:])
                a2a_in = st["a2a_in"]
                nc.sync.dma_start(
                    a2a_in[p * npr:(p + 1) * npr].transpose([1, 0, 2]),
                    ht.rearrange("p (s t) -> p s t", s=npr))
                if p == len(grp) - 1:
                    a2a_out = dr.tile([n_cores, CH, tokg], mm_dt,
                                      tag="a2aout", bufs=5,
                                      name=f"a2aout{b}_{gi}")
                    nc.gpsimd.collective_compute(
                        "AllToAll", mybir.AluOpType.bypass,
                        replica_groups=[list(range(n_cores))],
                        ins=[a2a_in.opt()], outs=[a2a_out.opt()])
                    tok0 = sum(512 * len(g) // n_cores for g in grps[:gi])
                    delay = RELEASE_FIRST if n_coll[0] == 0 else RELEASE_REST
                    n_coll[0] += 1
                    proj_ripe.append((it_counter[0] + delay,
                                      proj_job(b, a2a_out, tokg, tok0, gi)))

            # ---- the global iteration stream ----
            iters = [(b, qb, t) for b in range(B) for qb in range(QB)
                     for t in range(NK)]
            pending = None
            credit = 0.0
            for (b, qb, t) in iters:
                units[0] = 0
                pt = emit_sst_exp(b, qb, t)
                if pending is not None:
                    emit_hav(*pending)
                pending = (b, qb, t, pt)
                # release ripe proj jobs into the filler pool
                for ent in list(proj_ripe):
                    if ent[0] <= it_counter[0]:
                        fillq.append(ent[1])
                        proj_ripe.remove(ent)
                # budget: attention fixed cost ~3 slots vs exp cadence ~5.2
                credit += FILL_PER_ITER + 3.0 - units[0]
                credit = max(-6.0, min(8.0, credit))
                while credit >= 1.0 and fill_pull():
                    credit -= 1.0
                it_counter[0] += 1

            # ---- drain: final hav + chain, remaining filler, tail proj ----
            if pending is not None:
                emit_hav(*pending)
            while fill_pull():
                pass
            proj_ripe.sort(key=lambda e: e[0])
            while len(proj_ripe) > 1:
                g = proj_ripe.pop(0)[1]
                while next(g, None) is not None:
                    pass
            # keep the PE ramped across the last AllToAll's latency
            warmup(20)
            if proj_ripe:
                g = proj_ripe.pop(0)[1]
                while next(g, None) is not None:
                    pass

    nc.compile()
    return nc


def shard_inputs(x, qkv_w, proj_w, proj_b, n_cores=N_CORES, mm_dt=BF16):
    """Host-side sharding: pre-pack activations/weights, slice heads."""
    npdt = ml_dtypes.bfloat16 if mm_dt == BF16 else np.float32
    x = np.asarray(x)
    Bx, N, Cx = x.shape
    QB = N // 512
    xT = np.ascontiguousarray(np.transpose(x, (0, 2, 1))).astype(npdt)
    # xR[b, cb, r, k*512+c] = xT[b, 128k+r, 512cb+c]
    xR = np.ascontiguousarray(
        xT.reshape(Bx, KT_C, 128, QB, 512).transpose(0, 3, 2, 1, 4)
        .reshape(Bx, QB, 128, KT_C * 512))
    qkv_w = np.asarray(qkv_w)

    def pack(wT):  # [C, cols] -> [128, (C//128)*cols], k-tiles side by side
        cdim, cols = wT.shape
        return np.ascontiguousarray(
            wT.reshape(cdim // 128, 128, cols).transpose(1, 0, 2)
            .reshape(128, -1)).astype(npdt)

    pwT = pack(np.asarray(proj_w).T)
    pb = np.ascontiguousarray(np.asarray(proj_b)).astype(np.float32)
    nk = N // 128
    vonesc = np.ones((128, nk), dtype=npdt)
    in_maps = []
    for i in range(n_cores):
        cs = slice(CH * i, CH * i + CH)
        in_maps.append({
            "xR": xR,
            "wqT": pack(qkv_w[cs, :].T),
            "wkT": pack(qkv_w[C:][cs, :].T),
            "wvT": pack(qkv_w[2 * C:][cs, :].T),
            "pwT": pwT,
            "onesb": np.ones((1, 128), dtype=npdt),
            "pbb": pb.astype(npdt),
            "vonesc": vonesc,
        })
    return in_maps


def assemble_output(res, N, n_cores=N_CORES):
    QB = N // 512
    out = np.empty((B, N, C), dtype=np.float32)
    for i in range(n_cores):
        o = res.results[i]["out"]  # [B, TOKB, C]
        for b in range(B):
            grps = groups_for(b, QB)
            tok0 = 0
            base = 0
            for grp in grps:
                tokg = 512 * len(grp) // n_cores
                lo = base + tokg * i
                out[b, lo:lo + tokg, :] = o[b, tok0:tok0 + tokg]
                tok0 += tokg
                base += 512 * len(grp)
    return out


_NC_CACHE = {}


def _get_program(N, mm_dt=BF16):
    key = (N, str(mm_dt))
    if key not in _NC_CACHE:
        _NC_CACHE[key] = build_program(N=N, mm_dt=mm_dt)
    return _NC_CACHE[key]


def kernel(x, qkv_w, proj_w, proj_b):
    x = np.asarray(x)
    Bx, N, Cx = x.shape
    assert (Bx, Cx) == (B, C), (Bx, Cx)
    nc = _get_program(N)
    in_maps = shard_inputs(x, qkv_w, proj_w, proj_b)
    res = run_bass_kernel_spmd(nc, in_maps, list(range(N_CORES)))
    return assemble_output(res, N)


# revision 22
# speedup vs baseline: 1.2740x; 1.1447x over previous
"""Multi-head attention (B=2, N=2048, C=1024, H=16) on 8 TRN2 NeuronCores.

Sharding: tensor-parallel over heads (2 heads/core) for qkv+attention,
then AllToAll to token-shard the output projection.

v2: one globally software-pipelined schedule built around saturating the
Scalar (activation) engine, which has ~142us of exp work - the hard
bottleneck of this shard layout:
  - attention iterations (sst pair -> exp -> hav pair) start as soon as
    K(0)/Q(0)/V(0..3) exist (~8us), not after the full qkv phase
  - all remaining qkv matmuls (own batch, next batch) and the output
    projections are interleaved into the iteration stream as PE filler,
    budgeted so the PE always has work (keeps its p-state ramped at
    2.4GHz) but never starves ScalarE of sst inputs
  - sst for iteration i+1 is emitted before hav for iteration i so the
    PE never waits on exp latency
  - reciprocal_approx_fast instead of reciprocal (5x) for the softmax
    denominators; normalize chain kept off the critical path
  - per-qb scatter into the AllToAll buffer and per-group gather out of
    it are single strided DMAs (transposed APs) instead of 8-16 small
    serialized DMAs
  - PE warmup matmuls at t=0 and across the final AllToAll wait keep the
    p-state up where there is no real work to overlap
"""

import numpy as np
import ml_dtypes

import concourse.mybir as mybir
import concourse.tile as tile
from concourse import bacc
from concourse.bass_utils import run_bass_kernel_spmd

F32 = mybir.dt.float32
BF16 = mybir.dt.bfloat16
EXP = mybir.ActivationFunctionType.Exp

N_CORES = 8
B = 2
C = 1024
H = 16
D = 64
HPC = H // N_CORES          # heads per core
CH = HPC * D                # channels per core (128)
KT_C = C // 128             # contraction tiles (8)

RELEASE_FIRST = 34          # iters before first collective's proj releases
RELEASE_REST = 22           # iters for later batch-0 collectives
FILL_PER_ITER = 2.2         # filler matmuls per attention iteration
HAV_LAG = 6                 # max iterations hav may trail sst/exp


def groups_for(b, QB):
    g = [list(range(i, min(i + 2, QB))) for i in range(0, QB, 2)]
    if b == B - 1 and len(g) > 1:
        g = g[:-1] + [[q] for q in g[-1]]
    return g


def build_program(N=2048, n_cores=N_CORES, mm_dt=BF16):
    assert N % 512 == 0
    QB = N // 512            # 512-wide query/column blocks per batch
    NK = N // 128            # 128-row key tiles per batch
    scale = float(D) ** -0.5
    TOKB = N // n_cores

    nc = bacc.Bacc("TRN2", target_bir_lowering=False, debug=False,
                   num_devices=n_cores)

    # x host-packed per 512-column block: xR[b, cb, r, k*512+c]
    xR_d = nc.dram_tensor("xR", [B, QB, 128, KT_C * 512], mm_dt,
                          kind="ExternalInput")
    wqT_d = nc.dram_tensor("wqT", [128, KT_C * CH], mm_dt, kind="ExternalInput")
    wkT_d = nc.dram_tensor("wkT", [128, KT_C * CH], mm_dt, kind="ExternalInput")
    wvT_d = nc.dram_tensor("wvT", [128, KT_C * CH], mm_dt, kind="ExternalInput")
    pwT_d = nc.dram_tensor("pwT", [128, KT_C * C], mm_dt, kind="ExternalInput")
    onesb_d = nc.dram_tensor("onesb", [1, 128], mm_dt, kind="ExternalInput")
    pbb_d = nc.dram_tensor("pbb", [C], mm_dt, kind="ExternalInput")
    out_d = nc.dram_tensor("out", [B, TOKB, C], F32, kind="ExternalOutput")

    lp = nc.allow_low_precision("bf16 matmul pipeline")

    with tile.TileContext(nc) as tc:
        with (tc.tile_pool(name="sb", bufs=1) as sb,
              tc.tile_pool(name="ps", bufs=1, space="PSUM") as ps,
              tc.tile_pool(name="dr", bufs=1, space="DRAM") as dr,
              lp):
            # PSUM budget (8 banks): sst 2x2 + hav0 + hav1 + acc 2 (shared
            # by qkv chains, projections, warmup)

            # ---- weights / constants; DMA order: wk then x(0,0) first so
            # the first K chain can start ~4us in ----
            wkt = sb.tile([128, KT_C * CH], mm_dt, tag="wk", bufs=1, name="wk")
            nc.sync.dma_start(wkt[:], wkT_d.ap())

            xt = {}

            def load_x(b, cb):
                t = sb.tile([128, KT_C * 512], mm_dt, tag="xt", bufs=2 * QB,
                            name=f"x{b}_{cb}")
                nc.sync.dma_start(t[:], xR_d.ap()[b, cb])
                xt[(b, cb)] = t

            load_x(0, 0)
            wqt = sb.tile([128, KT_C * CH], mm_dt, tag="wq", bufs=1, name="wq")
            nc.sync.dma_start(wqt[:], wqT_d.ap())
            wvt = sb.tile([128, KT_C * CH], mm_dt, tag="wv", bufs=1, name="wv")
            nc.sync.dma_start(wvt[:], wvT_d.ap())
            onesb = sb.tile([1, 128], mm_dt, tag="onesb", bufs=1)
            nc.sync.dma_start(onesb[:], onesb_d.ap())
            pbb_sb = sb.tile([1, C], mm_dt, tag="pbb", bufs=1)
            nc.sync.dma_start(pbb_sb[:], pbb_d.ap().unsqueeze(0))
            # vau ones-stripes via gpsimd memset (a strided DMA for these
            # costs ~4us of sync-queue time each and delays the x loads)
            vau = [[] for _ in range(B)]
            for vb in range(B):
                for h in range(2):
                    t = sb.tile([128, 65 * NK], mm_dt, tag=f"vau{h}", bufs=2,
                                name=f"vau{vb}_{h}")
                    nc.gpsimd.memset(t[:, 64::65], 1.0)
                    vau[vb].append(t)
            for cb in range(1, QB):
                load_x(0, cb)
            for cb in range(QB):
                load_x(1, cb)
            pwt = sb.tile([128, KT_C * C], mm_dt, tag="pw", bufs=1, name="pw")
            nc.sync.dma_start(pwt[:], pwT_d.ap())

            wk = [wkt[:, CH * k:CH * k + CH] for k in range(KT_C)]
            wq = [wqt[:, CH * k:CH * k + CH] for k in range(KT_C)]
            wv = [wvt[:, CH * k:CH * k + CH] for k in range(KT_C)]
            pw = [pwt[:, C * k:C * k + C] for k in range(KT_C)]

            # ---- PE warmup: ramp the p-state before real work lands ----
            def warmup(n):
                wacc = ps.tile([128, 512], F32, tag="acc", bufs=2,
                               name="warm")
                for i in range(n):
                    nc.tensor.matmul(wacc[:], wkt[:, 0:128], wkt[:, 0:512],
                                     start=(i == 0), stop=(i == n - 1))

            warmup(10)

            # ---- per-batch qkv generators with milestone tracking ----
            kts = [{} for _ in range(B)]
            qts = [{} for _ in range(B)]
            done = [set() for _ in range(B)]

            def qkv_gen(b):
                def xs(cb, k, lo, w):
                    return xt[(b, cb)][:, 512 * k + lo:512 * k + lo + w]

                def kq_chain(which, qb, wlist, store, tag):
                    acc = ps.tile([128, 512], F32, tag="acc", bufs=2,
                                  name=f"{which}acc{b}_{qb}")
                    for k in range(KT_C):
                        nc.tensor.matmul(acc[:], wlist[k], xs(qb, k, 0, 512),
                                         start=(k == 0), stop=(k == KT_C - 1))
                        yield 1
                    tgt = sb.tile([128, 512], mm_dt, tag=tag, bufs=2 * QB + 1,
                                  name=f"{tag}{b}_{qb}")
                    nc.vector.tensor_copy(tgt[:], acc[:])
                    store[qb] = tgt
                    done[b].add((which, qb))

                def v_chain(tt):
                    cb, off = tt // 4, (tt % 4) * 128
                    acc = ps.tile([128, 128], F32, tag="acc", bufs=2,
                                  name=f"vacc{b}_{tt}")
                    for k in range(KT_C):
                        nc.tensor.matmul(acc[:], xs(cb, k, off, 128), wv[k],
                                         start=(k == 0), stop=(k == KT_C - 1))
                        yield 1
                    nc.vector.tensor_copy(
                        vau[b][0][:, 65 * tt:65 * tt + 64], acc[:, 0:64])
                    nc.vector.tensor_copy(
                        vau[b][1][:, 65 * tt:65 * tt + 64], acc[:, 64:128])
                    done[b].add(('V', tt))

                yield from kq_chain('K', 0, wk, kts[b], "kt")
                yield from kq_chain('Q', 0, wq, qts[b], "qt")
                for tt in range(4):
                    yield from v_chain(tt)
                for cb in range(1, QB):
                    yield from kq_chain('K', cb, wk, kts[b], "kt")
                    if cb == 1 and QB > 1:
                        yield from kq_chain('Q', 1, wq, qts[b], "qt")
                    for tt in range(4 * cb, 4 * cb + 4):
                        yield from v_chain(tt)
                for qb in range(2, QB):
                    yield from kq_chain('Q', qb, wq, qts[b], "qt")

            gens = [qkv_gen(b) for b in range(B)]

            units = [0]          # PE matmuls emitted this iteration

            def force(b, key):
                while key not in done[b]:
                    if next(gens[b], None) is None:
                        if key in done[b]:
                            break
                        raise RuntimeError(f"gen {b} dry before {key}")
                    units[0] += 1

            # ---- filler pool (priority order) + proj release queue ----
            fillq = list(gens)
            proj_ripe = []       # (release_at_iter, generator)
            it_counter = [0]

            def fill_pull():
                while fillq:
                    if next(fillq[0], None) is None:
                        fillq.pop(0)
                        continue
                    return True
                return False

            # ---- projection job per (batch, group) ----
            def proj_job(b, a2a_out, tokg, tok0, gi):
                pl = sb.tile([128, KT_C * tokg], mm_dt, tag="pl", bufs=2,
                             name=f"pl{b}_{gi}")
                nc.sync.dma_start(pl.rearrange("p (k t) -> p k t", k=KT_C),
                                  a2a_out.transpose([1, 0, 2]))
                for oh in range(2):
                    os_ = slice(512 * oh, 512 * oh + 512)
                    acc = ps.tile([128, 512], F32, tag="acc", bufs=2,
                                  name=f"pacc{b}_{gi}_{oh}")
                    for k in range(KT_C):
                        nc.tensor.matmul(
                            acc[0:tokg, :], pl[:, tokg * k:tokg * k + tokg],
                            pw[k][:, os_], start=(k == 0), stop=False)
                        yield 1
                    nc.tensor.matmul(acc[0:tokg, :], onesb[0:1, 0:tokg],
                                     pbb_sb[0:1, os_], start=False, stop=True)
                    yield 1
                    osb = sb.tile([128, 512], F32, tag="osb", bufs=2,
                                  name=f"osb{b}_{gi}_{oh}")
                    nc.vector.tensor_copy(osb[0:tokg, :], acc[0:tokg, :])
                    nc.sync.dma_start(
                        out_d.ap()[b, tok0:tok0 + tokg, os_], osb[0:tokg, :])

            # ---- attention building blocks ----
            hav = {}             # h -> current psum tile
            n_coll = [0]

            def emit_sst_exp(b, qb, t):
                force(b, ('K', t // 4))
                force(b, ('Q', qb))
                sst = ps.tile([128, 1024], F32, tag="sst", bufs=2,
                              name=f"sst{b}_{qb}_{t}")
                pt = sb.tile([128, 1024], mm_dt, tag="pt", bufs=8,
                             name=f"pt{b}_{qb}_{t}")
                kb, ko = t // 4, t % 4
                ks = slice(128 * ko, 128 * ko + 128)
                for h in range(2):
                    hs = slice(64 * h, 64 * h + 64)
                    nc.tensor.matmul(sst[:, 512 * h:512 * h + 512],
                                     kts[b][kb][hs, ks], qts[b][qb][hs, :],
                                     start=True, stop=True)
                units[0] += 1      # pair runs concurrently: one PE slot
                nc.scalar.activation(pt[:], sst[:], EXP, scale=scale)
                return pt

            grp_state = {}       # (b, gi) -> dict with a2a_in etc.

            def emit_hav(b, qb, t, pt):
                force(b, ('V', t))
                if t == 0:
                    for h in range(2):
                        hav[h] = ps.tile([65, 512], F32, tag=f"hav{h}",
                                         bufs=1, name=f"hav{b}_{qb}_{h}")
                for h in range(2):
                    nc.tensor.matmul(hav[h][:],
                                     vau[b][h][:, 65 * t:65 * t + 65],
                                     pt[:, 512 * h:512 * h + 512],
                                     start=(t == 0), stop=(t == NK - 1))
                units[0] += 2
                if t == NK - 1:
                    finish_qb(b, qb)

            def finish_qb(b, qb):
                grps = groups_for(b, QB)
                gi = next(i for i, g in enumerate(grps) if qb in g)
                grp = grps[gi]
                p = grp.index(qb)
                tokg = 512 * len(grp) // n_cores
                npr = 512 // tokg
                st = grp_state.setdefault((b, gi), {})
                if "a2a_in" not in st:
                    st["a2a_in"] = dr.tile([n_cores, CH, tokg], mm_dt,
                                           tag="a2ain", bufs=5,
                                           name=f"a2ain{b}_{gi}")
                # normalize: h/denom with approx reciprocal, then one
                # strided scatter DMA into the group's AllToAll buffer
                ht = sb.tile([128, 512], mm_dt, tag="ht", bufs=2,
                             name=f"ht{b}_{qb}")
                for h in range(2):
                    # denominator row to a partition-0 tile: the custom-DVE
                    # approx reciprocal mishandles base_partition != 0
                    dnm = sb.tile([1, 512], F32, tag=f"dnm{h}", bufs=2,
                                  name=f"dnm{b}_{qb}_{h}")
                    nc.vector.tensor_copy(dnm[:], hav[h][64:65, :])
                    hcp = sb.tile([64, 512], F32, tag=f"hcp{h}", bufs=2,
                                  name=f"hcp{b}_{qb}_{h}")
                    nc.vector.tensor_copy(hcp[:], hav[h][0:64, :])
                    nrr = sb.tile([1, 512], F32, tag=f"nrr{h}", bufs=2,
                                  name=f"nrr{b}_{qb}_{h}")
                    nc.vector.reciprocal_approx_fast(nrr[:], dnm[:])
                    bcs = sb.tile([64, 512], F32, tag=f"bcs{h}", bufs=2,
                                  name=f"bcs{b}_{qb}_{h}")
                    nc.gpsimd.partition_broadcast(bcs[:], nrr[:])
                    nc.vector.tensor_mul(ht[64 * h:64 * h + 64, :],
                                         hcp[:], bcs[:])
                a2a_in = st["a2a_in"]
                nc.sync.dma_start(
                    a2a_in[p * npr:(p + 1) * npr].transpose([1, 0, 2]),
                    ht.rearrange("p (s t) -> p s t", s=npr))
                if p == len(grp) - 1:
                    a2a_out = dr.tile([n_cores, CH, tokg], mm_dt,
                                      tag="a2aout", bufs=5,
                                      name=f"a2aout{b}_{gi}")
                    nc.gpsimd.collective_compute(
                        "AllToAll", mybir.AluOpType.bypass,
                        replica_groups=[list(range(n_cores))],
                        ins=[a2a_in.opt()], outs=[a2a_out.opt()])
                    tok0 = sum(512 * len(g) // n_cores for g in grps[:gi])
                    # last batch's projs run in the drain, where they double
                    # as PE filler across the final collectives' latency
                    if b == B - 1:
                        delay = 10 ** 9
                    elif n_coll[0] == 0:
                        delay = RELEASE_FIRST
                    else:
                        delay = RELEASE_REST
                    n_coll[0] += 1
                    proj_ripe.append((it_counter[0] + delay,
                                      proj_job(b, a2a_out, tokg, tok0, gi)))

            # ---- the global iteration stream ----
            iters = [(b, qb, t) for b in range(B) for qb in range(QB)
                     for t in range(NK)]
            pending = []
            credit = 0.0
            for (b, qb, t) in iters:
                units[0] = 0
                pt = emit_sst_exp(b, qb, t)
                pending.append((b, qb, t, pt))
                # emit hav once its V tile exists, or when the lag cap hits;
                # deferring spreads the V-chain crunch past the first qb
                while pending and (len(pending) > HAV_LAG
                                   or ('V', pending[0][2]) in done[pending[0][0]]):
                    emit_hav(*pending.pop(0))
                # release ripe proj jobs into the filler pool
                for ent in list(proj_ripe):
                    if ent[0] <= it_counter[0]:
                        fillq.append(ent[1])
                        proj_ripe.remove(ent)
                # budget: attention fixed cost ~3 slots vs exp cadence ~5.2
                credit += FILL_PER_ITER + 3.0 - units[0]
                credit = max(-6.0, min(8.0, credit))
                while credit >= 1.0 and fill_pull():
                    credit -= 1.0
                it_counter[0] += 1

            # ---- drain: final havs + chain, remaining filler, tail projs ----
            while pending:
                emit_hav(*pending.pop(0))
            while fill_pull():
                pass
            proj_ripe.sort(key=lambda e: e[0])
            jobs = [g for _, g in proj_ripe]
            proj_ripe.clear()
            for g in jobs[:-1]:
                while next(g, None) is not None:
                    pass
            # keep the PE ramped across the last AllToAll's latency
            warmup(10)
            for g in jobs[-1:]:
                while next(g, None) is not None:
                    pass

    nc.compile()
    return nc


def shard_inputs(x, qkv_w, proj_w, proj_b, n_cores=N_CORES, mm_dt=BF16):
    """Host-side sharding: pre-pack activations/weights, slice heads."""
    npdt = ml_dtypes.bfloat16 if mm_dt == BF16 else np.float32
    x = np.asarray(x)
    Bx, N, Cx = x.shape
    QB = N // 512
    xT = np.ascontiguousarray(np.transpose(x, (0, 2, 1))).astype(npdt)
    # xR[b, cb, r, k*512+c] = xT[b, 128k+r, 512cb+c]
    xR = np.ascontiguousarray(
        xT.reshape(Bx, KT_C, 128, QB, 512).transpose(0, 3, 2, 1, 4)
        .reshape(Bx, QB, 128, KT_C * 512))
    qkv_w = np.asarray(qkv_w)

    def pack(wT):  # [C, cols] -> [128, (C//128)*cols], k-tiles side by side
        cdim, cols = wT.shape
        return np.ascontiguousarray(
            wT.reshape(cdim // 128, 128, cols).transpose(1, 0, 2)
            .reshape(128, -1)).astype(npdt)

    pwT = pack(np.asarray(proj_w).T)
    pb = np.ascontiguousarray(np.asarray(proj_b)).astype(np.float32)
    in_maps = []
    for i in range(n_cores):
        cs = slice(CH * i, CH * i + CH)
        in_maps.append({
            "xR": xR,
            "wqT": pack(qkv_w[cs, :].T),
            "wkT": pack(qkv_w[C:][cs, :].T),
            "wvT": pack(qkv_w[2 * C:][cs, :].T),
            "pwT": pwT,
            "onesb": np.ones((1, 128), dtype=npdt),
            "pbb": pb.astype(npdt),
        })
    return in_maps


def assemble_output(res, N, n_cores=N_CORES):
    QB = N // 512
    out = np.empty((B, N, C), dtype=np.float32)
    for i in range(n_cores):
        o = res.results[i]["out"]  # [B, TOKB, C]
        for b in range(B):
            grps = groups_for(b, QB)
            tok0 = 0
            base = 0
            for grp in grps:
                tokg = 512 * len(grp) // n_cores
                lo = base + tokg * i
                out[b, lo:lo + tokg, :] = o[b, tok0:tok0 + tokg]
                tok0 += tokg
                base += 512 * len(grp)
    return out


_NC_CACHE = {}


def _get_program(N, mm_dt=BF16):
    key = (N, str(mm_dt))
    if key not in _NC_CACHE:
        _NC_CACHE[key] = build_program(N=N, mm_dt=mm_dt)
    return _NC_CACHE[key]


def kernel(x, qkv_w, proj_w, proj_b):
    x = np.asarray(x)
    Bx, N, Cx = x.shape
    assert (Bx, Cx) == (B, C), (Bx, Cx)
    nc = _get_program(N)
    in_maps = shard_inputs(x, qkv_w, proj_w, proj_b)
    res = run_bass_kernel_spmd(nc, in_maps, list(range(N_CORES)))
    return assemble_output(res, N)
